# revision 39
# baseline (speedup 1.0000x reference)
"""LlamaAttention (GQA, no mask) on 8 Trainium2 NeuronCores.

Sharding: 8 cores = 2 (batch) x 4 (head groups of 8 heads / 2 KV heads).
Per core (bf16 compute, fp32 accumulation):
  qT  = (x_b @ wq_g)^T            [512, 2048]   (head dims on partitions)
  kTd = (x_b @ wk_g)^T duplicated [128, 2, 2048]
  v   = x_b @ wv_g (+ ones col)   [2048, 2, 65]
  attention per head pair: sT[k,q] matmuls -> exp on ACT -> flipped AV
    matmuls out[q-tile 128, 65] (full-M: half the PE streaming of the
    [65, q] orientation) accumulated in a packed 3-bank PSUM tile ->
    bulk copy to SBUF -> per-partition reciprocal + mul normalize -> PE
    transpose (identity matmul) back to at^T[d, q] for o_proj.
  out_partial = at @ wo_g         [2048, 2048] fp32
Host sums the 4 head-group partials per batch.

Scheduling: the exp stream on ACT (256 x [128,1024], ~266us) is the body
pacer; every other PE matmul (v/q/o projections, k second half) is
emitted as ~2048-cycle quanta inside the attention kc loop so the PE
array fills ACT-wait slack. Normalize+transpose of pair p is emitted
inside pair p+1's kc loop. xt chunk DMAs are split 4-way so early
contraction chunks land early and the prologue projections chase DMA
arrivals.
"""

import numpy as np
import ml_dtypes

S = 2048          # sequence length
D = 2048          # model dim
HD = 64           # head dim
GH = 8            # heads per core
QC = GH * HD      # 512 q cols per core
KVC = 128         # kv cols per core (2 kv heads)
DC = D // 128     # 16 contraction chunks
SC = S // 128     # 16 seq chunks
SCALE = HD ** -0.5

_CACHE = {}


def _build():
    import concourse.bass as bass
    import concourse.mybir as mybir
    import concourse.tile as tile
    from concourse import bacc, masks

    f32 = mybir.dt.float32
    bf16 = mybir.dt.bfloat16
    Exp = mybir.ActivationFunctionType.Exp

    nc = bacc.Bacc("TRN2", target_bir_lowering=False, debug=False, num_devices=8)

    # weights come host-pre-chunked to partition-major [128, dc, n] layouts
    # so every DMA moves >=4KB contiguous runs (half-rate below 512B); wq is
    # split so pair 0's columns can be prioritized on the serial DMA bus
    xt = nc.dram_tensor("xt", [D, S], bf16, kind="ExternalInput").ap()
    wq0 = nc.dram_tensor("wq0", [128, DC, 128], bf16, kind="ExternalInput").ap()
    wqr = nc.dram_tensor("wqr", [128, DC, 384], bf16, kind="ExternalInput").ap()
    wk = nc.dram_tensor("wk", [128, DC, KVC], bf16, kind="ExternalInput").ap()
    wv = nc.dram_tensor("wv", [128, DC, KVC], bf16, kind="ExternalInput").ap()
    wo = nc.dram_tensor("wo", [128, QC // 128, D], bf16, kind="ExternalInput").ap()
    out = nc.dram_tensor("out", [S, D], f32, kind="ExternalOutput").ap()

    with tile.TileContext(nc) as tc:
        with tc.tile_pool(name="const", bufs=1) as const, \
             tc.tile_pool(name="sps", bufs=2, space="PSUM") as sps, \
             tc.tile_pool(name="pjp", bufs=1, space="PSUM") as pjp, \
             tc.tile_pool(name="oap", bufs=1, space="PSUM") as oap, \
             tc.tile_pool(name="ev", bufs=2) as ev, \
             tc.tile_pool(name="ppool", bufs=3) as ppool, \
             tc.tile_pool(name="dpool", bufs=2) as dpool:

            # resident inputs, issued on ONE queue (SP) in priority order so
            # the serial DMA bus delivers exactly what the prologue needs
            # first: wk + wq(pair0 cols) -> xt chunks (the k/q projections
            # chase the per-chunk arrivals) -> wv -> wq rest -> wo
            wk_all = const.tile([128, DC, KVC], bf16, tag="wk_all")
            nc.sync.dma_start(out=wk_all[:], in_=wk)
            wq0_sb = const.tile([128, DC, 128], bf16, tag="wq0_sb")
            nc.sync.dma_start(out=wq0_sb[:], in_=wq0)
            xt_all = const.tile([128, DC, S], bf16, tag="xt_all")
            xt_re = xt.rearrange("(c p) s -> p c s", p=128)
            for dc in range(DC):
                nc.sync.dma_start(out=xt_all[:, dc, :], in_=xt_re[:, dc, :])
            # wv/wqr/wo DMAs are issued AFTER the ktd dup DMAs below so the
            # dups (needed by the first scores matmul) aren't stuck behind
            # them on the serial DMA bus
            wv_all = const.tile([128, DC, KVC], bf16, tag="wv_all")
            wqr_sb = const.tile([128, DC, 384], bf16, tag="wqr_sb")
            wo_all = const.tile([128, QC // 128, D], bf16, tag="wo_all")

            def wq_sl(qm, dc):
                if qm == 0:
                    return wq0_sb[:, dc, :]
                return wqr_sb[:, dc, (qm - 1) * 128:qm * 128]

            ident = const.tile([128, 128], bf16, tag="ident")
            masks.make_identity(nc, ident[:])

            # persistent intermediates
            qpair = const.tile([128, 4, S], bf16, tag="qpair")     # q^T
            ktd = const.tile([128, 2, S], bf16, tag="ktd")         # k^T dup per kv head
            vv = const.tile([128, SC, 130], bf16, tag="vv")        # v (+ones cols)
            at = const.tile([128, 4, S], bf16, tag="at")           # attn out^T

            nc.vector.memset(vv[:, :, 64:65], 1.0)
            nc.vector.memset(vv[:, :, 129:130], 1.0)

            def mmacc(out_t, lhsT, rhs, width, start, stop):
                # moving-operand ISA limit is 512: split wide matmuls
                for o in range(0, width, 512):
                    nc.tensor.matmul(out_t[:, o:o + 512], lhsT,
                                     rhs[:, o:o + 512], start=start, stop=stop)

            # ---------------- prologue: k + q(pair0) chase the xt DMAs -----
            # k nb0/nb1 in the two sps slots, q pair0 half0 in pj, half1 in
            # the (otherwise idle) o_all psum: all four accumulate per-dc as
            # the xt chunks land.
            k_ps = [sps.tile([128, 1024], f32, tag="s_ps", name=f"k_ps{nb}")
                    for nb in range(2)]
            q_pj = pjp.tile([128, 512], f32, tag="pj")
            o_pro = oap.tile([128, 3, 512], f32, tag="o_all")
            for dc in range(DC):
                for nb in range(2):
                    mmacc(k_ps[nb], wk_all[:, dc, :],
                          xt_all[:, dc, nb * 1024:(nb + 1) * 1024], 1024,
                          (dc == 0), (dc == DC - 1))
                nc.tensor.matmul(q_pj[:], wq_sl(0, dc),
                                 xt_all[:, dc, 0:512],
                                 start=(dc == 0), stop=(dc == DC - 1))
                nc.tensor.matmul(o_pro[:, 0, :], wq_sl(0, dc),
                                 xt_all[:, dc, 512:1024],
                                 start=(dc == 0), stop=(dc == DC - 1))
            for nb in range(2):
                kt_sb = ev.tile([128, 1024], bf16, tag="kt_sb")
                nc.vector.tensor_copy(kt_sb[:], k_ps[nb][:])
                sl = slice(nb * 1024, (nb + 1) * 1024)
                nc.sync.dma_start(out=ktd[0:64, 0, sl], in_=kt_sb[0:64, :])
                nc.sync.dma_start(out=ktd[64:128, 0, sl], in_=kt_sb[0:64, :])
                nc.sync.dma_start(out=ktd[0:64, 1, sl], in_=kt_sb[64:128, :])
                nc.sync.dma_start(out=ktd[64:128, 1, sl], in_=kt_sb[64:128, :])
            nc.vector.tensor_copy(qpair[:, 0, 0:512], q_pj[:])
            nc.vector.tensor_copy(qpair[:, 0, 512:1024], o_pro[:, 0, :])
            # remaining weights, behind the ktd dups on the bus
            nc.sync.dma_start(out=wv_all[:], in_=wv)
            nc.sync.dma_start(out=wqr_sb[:], in_=wqr)
            nc.sync.dma_start(out=wo_all[:], in_=wo)

            # ---------------- filler work units (~2048 PE cycle quanta) ----
            def v_chunk(sc):
                def run():
                    ps = pjp.tile([128, 512], f32, tag="pj")
                    for dc in range(DC):
                        nc.tensor.matmul(ps[:, 0:KVC],
                                         xt_all[:, dc, sc * 128:(sc + 1) * 128],
                                         wv_all[:, dc, :],
                                         start=(dc == 0), stop=(dc == DC - 1))
                    yield 2048
                    nc.vector.tensor_copy(vv[:, sc, 0:64], ps[:, 0:64])
                    nc.vector.tensor_copy(vv[:, sc, 65:129], ps[:, 64:128])
                return run

            def q_half(qm, jbb, h):
                def run():
                    ps = pjp.tile([128, 512], f32, tag="pj")
                    sl = slice(jbb * 1024 + h * 512, jbb * 1024 + (h + 1) * 512)
                    for dq in range(0, DC, 4):
                        for dc in range(dq, dq + 4):
                            nc.tensor.matmul(ps[:], wq_sl(qm, dc),
                                             xt_all[:, dc, sl],
                                             start=(dc == 0), stop=(dc == DC - 1))
                        yield 2048
                    nc.vector.tensor_copy(qpair[:, qm, sl], ps[:])
                return run

            def o_piece(sm, pc):
                def run():
                    ps = pjp.tile([128, 512], f32, tag="pj")
                    for cc in range(4):
                        nc.tensor.matmul(ps[:], at[:, cc, sm * 128:(sm + 1) * 128],
                                         wo_all[:, cc, pc * 512:(pc + 1) * 512],
                                         start=(cc == 0), stop=(cc == 3))
                    yield 2048
                    o_sb = ev.tile([128, 512], f32, tag="o_sb")
                    nc.vector.tensor_copy(o_sb[:], ps[:])
                    rs = slice(sm * 128, (sm + 1) * 128)
                    nc.sync.dma_start(out=out[rs, pc * 512:(pc + 1) * 512], in_=o_sb[:])
                return run

            # deadline-aware FIFO of filler generators
            queue = []            # [name, ...]
            gens = {}             # name -> generator factory (unstarted)
            started = {}          # name -> running generator
            budget = [0]

            def push(name, factory):
                queue.append(name)
                gens[name] = factory

            def _resume(name):
                g = started.get(name)
                if g is None:
                    g = started[name] = gens.pop(name)()
                try:
                    return next(g)
                except StopIteration:
                    del started[name]
                    queue.remove(name)
                    return None

            def fill(cycles):
                budget[0] += cycles
                while budget[0] > 0 and queue:
                    cost = _resume(queue[0])
                    if cost is not None:
                        budget[0] -= cost

            def require(name):
                # force a unit to finish emission now (deadline)
                while name in queue:
                    _resume(name)

            def drain_all():
                while queue:
                    _resume(queue[0])

            for sc in range(3):
                for _ in v_chunk(sc)():
                    pass
            for sc in range(3, SC):
                push(f"v{sc}", v_chunk(sc))
            push("q1h0", q_half(1, 0, 0))
            push("q1h1", q_half(1, 0, 1))
            releases = {
                0: [("q2h0", q_half(2, 0, 0)), ("q2h1", q_half(2, 0, 1))],
                1: [("q3h0", q_half(3, 0, 0)), ("q3h1", q_half(3, 0, 1))],
                2: [("q4h0", q_half(0, 1, 0)), ("q4h1", q_half(0, 1, 1))],
                3: [("q5h0", q_half(1, 1, 0)), ("q5h1", q_half(1, 1, 1))],
                4: [("q6h0", q_half(2, 1, 0)), ("q6h1", q_half(2, 1, 1)),
                    ("q7h0", q_half(3, 1, 0)), ("q7h1", q_half(3, 1, 1))],
            }

            # packed AV accumulator slots: 18 x [128, 65] f32 in 3 PSUM banks
            def o_slot(t, s, lo, hi):
                b, i = s // 6, s % 6
                return t[:, b, 85 * i + lo:85 * i + hi]

            def do_av(o_all, kc, p_A, p_B, kv):
                for h2, p in ((0, p_A), (1, p_B)):
                    for qt in range(8):
                        dst = o_slot(o_all, h2 * 8 + qt, 0, 65)
                        nc.tensor.matmul(dst, p[:, qt * 128:(qt + 1) * 128],
                                         vv[:, kc, kv * 65:kv * 65 + 65],
                                         start=(kc == 0), stop=(kc == SC - 1))

            def nt_unit(o_st, qm, jbb, qt):
                # normalize one q-tile of both heads (DVE) + transpose (PE)
                def run():
                    at_n2 = ev.tile([128, 128], bf16, tag="at_n2")
                    for h2 in range(2):
                        num = o_slot(o_st, h2 * 8 + qt, 0, 64)
                        den = o_slot(o_st, h2 * 8 + qt, 64, 65)
                        rden = dpool.tile([128, 1], f32, tag="rden")
                        nc.vector.reciprocal(rden[:], den)
                        nc.vector.tensor_scalar_mul(
                            at_n2[:, h2 * 64:(h2 + 1) * 64], num, rden[:])
                    tp = pjp.tile([128, 128], bf16, tag="pj")
                    nc.tensor.matmul(tp[:], at_n2[:], ident[:], is_transpose=True)
                    nc.vector.tensor_copy(
                        at[:, qm, jbb * 1024 + qt * 128:jbb * 1024 + (qt + 1) * 128],
                        tp[:])
                    yield 512
                return run

            # ------------- fused attention, per (jb, qm) head pair ----------
            for jb in range(2):
                qsl = slice(jb * 1024, (jb + 1) * 1024)
                for qm in range(4):
                    pair = jb * 4 + qm
                    for name, factory in releases.get(pair, []):
                        push(name, factory)
                    require(f"q{pair}h0")
                    require(f"q{pair}h1")
                    kv = qm // 2
                    o_all = oap.tile([128, 3, 512], f32, tag="o_all")
                    prev = None
                    for kc in range(SC):
                        if prev is not None:
                            require(f"v{prev[0]}")
                            do_av(o_all, *prev, kv)
                        ksl = slice(kc * 128, (kc + 1) * 128)
                        ps_A = sps.tile([128, 1024], f32, tag="s_ps")
                        mmacc(ps_A, ktd[0:64, kv, ksl],
                              qpair[0:64, qm, qsl], 1024, True, True)
                        p_A = ppool.tile([128, 1024], bf16, tag="p_A")
                        nc.scalar.activation(p_A[:], ps_A[:], Exp, scale=SCALE)
                        ps_B = sps.tile([128, 1024], f32, tag="s_ps")
                        mmacc(ps_B, ktd[64:128, kv, ksl],
                              qpair[64:128, qm, qsl], 1024, True, True)
                        p_B = ppool.tile([128, 1024], bf16, tag="p_B")
                        nc.scalar.activation(p_B[:], ps_B[:], Exp, scale=SCALE)
                        # taper the filler near the pair end so PE is caught
                        # up when the boundary-critical AV/scores arrive
                        fill(3584 if kc < 12 else 1024)
                        prev = (kc, p_A, p_B)
                    require(f"v{prev[0]}")
                    do_av(o_all, *prev, kv)

                    # free the AV psum banks right away; normalize+transpose
                    # run from the SBUF staging copy, spread over the next
                    # pair's filler slots
                    o_st = ev.tile([128, 3, 512], f32, tag="o_st")
                    nc.vector.tensor_copy(o_st[:], o_all[:])
                    for qt in range(8):
                        push(f"nt{pair}_{qt}", nt_unit(o_st, qm, jb, qt))

                # after jb=0's pairs, at[:, :, 0:1024] completes during pair
                # 4's nt units: release its o_proj pieces from pair 4 on
                # (o_piece(sm) only reads at[:, :, sm*128:...], whose nt unit
                # qt=sm lands before the piece pops from the queue)
                if jb == 0:
                    rel = releases.setdefault(4, [])
                    for sm in range(8):
                        for pc in range(4):
                            rel.append((f"o{sm}_{pc}", o_piece(sm, pc)))

            # ------------- epilogue -----------------------------------------
            # drain remaining queue (includes the last pairs' normalize+
            # transpose units), then jb=1's o_proj on the freed scores pool
            drain_all()
            for sm in range(8, 16):
                for nb in range(2):
                    ps = sps.tile([128, 1024], f32, tag="s_ps")
                    for cc in range(4):
                        mmacc(ps, at[:, cc, sm * 128:(sm + 1) * 128],
                              wo_all[:, cc, nb * 1024:(nb + 1) * 1024], 1024,
                              (cc == 0), (cc == 3))
                    o_sb = ev.tile([128, 1024], f32, tag="o_sb2", bufs=4)
                    nc.vector.tensor_copy(o_sb[:], ps[:])
                    rs = slice(sm * 128, (sm + 1) * 128)
                    nc.sync.dma_start(out=out[rs, nb * 1024:(nb + 1) * 1024],
                                      in_=o_sb[:])

    nc.compile()
    return nc


def _get_nc():
    if "nc" not in _CACHE:
        _CACHE["nc"] = _build()
    return _CACHE["nc"]


def kernel(x, wq, wk, wv, wo):
    from concourse.bass_utils import run_bass_kernel_spmd

    bf16 = ml_dtypes.bfloat16
    nc = _get_nc()

    def chunk_d(a):
        # [D, n] -> [128, DC, n]: partition-major contraction chunks
        n = a.shape[1]
        return np.ascontiguousarray(
            a.reshape(DC, 128, n).transpose(1, 0, 2)).astype(bf16)

    in_maps = []
    for core in range(8):
        b, g = core // 4, core % 4
        wq_g = np.asarray(wq)[:, g * QC:(g + 1) * QC]
        wo_g = np.asarray(wo)[g * QC:(g + 1) * QC, :]
        in_maps.append({
            "xt": np.ascontiguousarray(np.asarray(x)[b].T).astype(bf16),
            "wq0": chunk_d(wq_g[:, 0:128]),
            "wqr": chunk_d(wq_g[:, 128:QC]),
            "wk": chunk_d(np.asarray(wk)[:, g * KVC:(g + 1) * KVC]),
            "wv": chunk_d(np.asarray(wv)[:, g * KVC:(g + 1) * KVC]),
            "wo": np.ascontiguousarray(
                wo_g.reshape(QC // 128, 128, D).transpose(1, 0, 2)).astype(bf16),
        })

    res = run_bass_kernel_spmd(nc, in_maps, core_ids=list(range(8)))
    outs = [res.results[c]["out"] for c in range(8)]
    full = np.empty((2, S, D), np.float32)
    full[0] = outs[0] + outs[1] + outs[2] + outs[3]
    full[1] = outs[4] + outs[5] + outs[6] + outs[7]
    return full


# revision 41
# speedup vs baseline: 1.0037x; 1.0037x over previous
"""LlamaAttention (GQA, no mask) on 8 Trainium2 NeuronCores.

Sharding: 8 cores = 2 (batch) x 4 (head groups of 8 heads / 2 KV heads).
Per core (bf16 compute, fp32 accumulation):
  qT  = (x_b @ wq_g)^T            [512, 2048]   (head dims on partitions)
  kTd = (x_b @ wk_g)^T duplicated [128, 2, 2048]
  v   = x_b @ wv_g (+ ones col)   [2048, 2, 65]
  attention per head pair: sT[k,q] matmuls -> exp on ACT -> flipped AV
    matmuls out[q-tile 128, 65] (full-M: half the PE streaming of the
    [65, q] orientation) accumulated in a packed 3-bank PSUM tile ->
    bulk copy to SBUF -> per-partition reciprocal + mul normalize -> PE
    transpose (identity matmul) back to at^T[d, q] for o_proj.
  out_partial = at @ wo_g         [2048, 2048] fp32
Host sums the 4 head-group partials per batch.

Scheduling: the exp stream on ACT (256 x [128,1024], ~266us) is the body
pacer; every other PE matmul (v/q/o projections, k second half) is
emitted as ~2048-cycle quanta inside the attention kc loop so the PE
array fills ACT-wait slack. Normalize+transpose of pair p is emitted
inside pair p+1's kc loop. xt chunk DMAs are split 4-way so early
contraction chunks land early and the prologue projections chase DMA
arrivals.
"""

import numpy as np
import ml_dtypes

S = 2048          # sequence length
D = 2048          # model dim
HD = 64           # head dim
GH = 8            # heads per core
QC = GH * HD      # 512 q cols per core
KVC = 128         # kv cols per core (2 kv heads)
DC = D // 128     # 16 contraction chunks
SC = S // 128     # 16 seq chunks
SCALE = HD ** -0.5

_CACHE = {}


def _build():
    import concourse.bass as bass
    import concourse.mybir as mybir
    import concourse.tile as tile
    from concourse import bacc, masks

    f32 = mybir.dt.float32
    bf16 = mybir.dt.bfloat16
    Exp = mybir.ActivationFunctionType.Exp

    nc = bacc.Bacc("TRN2", target_bir_lowering=False, debug=False, num_devices=8)

    # weights come host-pre-chunked to partition-major [128, dc, n] layouts
    # so every DMA moves >=4KB contiguous runs (half-rate below 512B); wq is
    # split so pair 0's columns can be prioritized on the serial DMA bus
    xt = nc.dram_tensor("xt", [D, S], bf16, kind="ExternalInput").ap()
    wq0 = nc.dram_tensor("wq0", [128, DC, 128], bf16, kind="ExternalInput").ap()
    wqr = nc.dram_tensor("wqr", [128, DC, 384], bf16, kind="ExternalInput").ap()
    wk = nc.dram_tensor("wk", [128, DC, KVC], bf16, kind="ExternalInput").ap()
    wv = nc.dram_tensor("wv", [128, DC, KVC], bf16, kind="ExternalInput").ap()
    wo = nc.dram_tensor("wo", [128, QC // 128, D], bf16, kind="ExternalInput").ap()
    out = nc.dram_tensor("out", [S, D], f32, kind="ExternalOutput").ap()

    with tile.TileContext(nc) as tc:
        with tc.tile_pool(name="const", bufs=1) as const, \
             tc.tile_pool(name="sps", bufs=2, space="PSUM") as sps, \
             tc.tile_pool(name="pjp", bufs=1, space="PSUM") as pjp, \
             tc.tile_pool(name="oap", bufs=1, space="PSUM") as oap, \
             tc.tile_pool(name="ev", bufs=2) as ev, \
             tc.tile_pool(name="ppool", bufs=3) as ppool, \
             tc.tile_pool(name="dpool", bufs=2) as dpool:

            # resident inputs, issued on ONE queue (SP) in priority order so
            # the serial DMA bus delivers exactly what the prologue needs
            # first: wk + wq(pair0 cols) -> xt chunks (the k/q projections
            # chase the per-chunk arrivals) -> wv -> wq rest -> wo
            wk_all = const.tile([128, DC, KVC], bf16, tag="wk_all")
            nc.sync.dma_start(out=wk_all[:], in_=wk)
            wq0_sb = const.tile([128, DC, 128], bf16, tag="wq0_sb")
            nc.sync.dma_start(out=wq0_sb[:], in_=wq0)
            xt_all = const.tile([128, DC, S], bf16, tag="xt_all")
            xt_re = xt.rearrange("(c p) s -> p c s", p=128)
            for dc in range(DC):
                nc.sync.dma_start(out=xt_all[:, dc, :], in_=xt_re[:, dc, :])
            # wv/wqr/wo DMAs are issued AFTER the ktd dup DMAs below so the
            # dups (needed by the first scores matmul) aren't stuck behind
            # them on the serial DMA bus
            wv_all = const.tile([128, DC, KVC], bf16, tag="wv_all")
            wqr_sb = const.tile([128, DC, 384], bf16, tag="wqr_sb")
            wo_all = const.tile([128, QC // 128, D], bf16, tag="wo_all")

            def wq_sl(qm, dc):
                if qm == 0:
                    return wq0_sb[:, dc, :]
                return wqr_sb[:, dc, (qm - 1) * 128:qm * 128]

            ident = const.tile([128, 128], bf16, tag="ident")
            masks.make_identity(nc, ident[:])

            # persistent intermediates
            qpair = const.tile([128, 4, S], bf16, tag="qpair")     # q^T
            ktd = const.tile([128, 2, S], bf16, tag="ktd")         # k^T dup per kv head
            vv = const.tile([128, SC, 130], bf16, tag="vv")        # v (+ones cols)
            at = const.tile([128, 4, S], bf16, tag="at")           # attn out^T

            nc.vector.memset(vv[:, :, 64:65], 1.0)
            nc.vector.memset(vv[:, :, 129:130], 1.0)

            def mmacc(out_t, lhsT, rhs, width, start, stop):
                # moving-operand ISA limit is 512: split wide matmuls
                for o in range(0, width, 512):
                    nc.tensor.matmul(out_t[:, o:o + 512], lhsT,
                                     rhs[:, o:o + 512], start=start, stop=stop)

            # ---------------- prologue: k + q(pair0) chase the xt DMAs -----
            # k nb0/nb1 in the two sps slots, q pair0 half0 in pj, half1 in
            # the (otherwise idle) o_all psum: all four accumulate per-dc as
            # the xt chunks land.
            k_ps = [sps.tile([128, 1024], f32, tag="s_ps", name=f"k_ps{nb}")
                    for nb in range(2)]
            q_pj = pjp.tile([128, 512], f32, tag="pj")
            o_pro = oap.tile([128, 3, 512], f32, tag="o_all")
            for dc in range(DC):
                for nb in range(2):
                    mmacc(k_ps[nb], wk_all[:, dc, :],
                          xt_all[:, dc, nb * 1024:(nb + 1) * 1024], 1024,
                          (dc == 0), (dc == DC - 1))
                nc.tensor.matmul(q_pj[:], wq_sl(0, dc),
                                 xt_all[:, dc, 0:512],
                                 start=(dc == 0), stop=(dc == DC - 1))
                nc.tensor.matmul(o_pro[:, 0, :], wq_sl(0, dc),
                                 xt_all[:, dc, 512:1024],
                                 start=(dc == 0), stop=(dc == DC - 1))
            for nb in range(2):
                kt_sb = ev.tile([128, 1024], bf16, tag="kt_sb")
                nc.vector.tensor_copy(kt_sb[:], k_ps[nb][:])
                sl = slice(nb * 1024, (nb + 1) * 1024)
                nc.sync.dma_start(out=ktd[0:64, 0, sl], in_=kt_sb[0:64, :])
                nc.sync.dma_start(out=ktd[64:128, 0, sl], in_=kt_sb[0:64, :])
                nc.sync.dma_start(out=ktd[0:64, 1, sl], in_=kt_sb[64:128, :])
                nc.sync.dma_start(out=ktd[64:128, 1, sl], in_=kt_sb[64:128, :])
            nc.vector.tensor_copy(qpair[:, 0, 0:512], q_pj[:])
            nc.vector.tensor_copy(qpair[:, 0, 512:1024], o_pro[:, 0, :])
            # remaining weights, behind the ktd dups on the bus
            nc.sync.dma_start(out=wv_all[:], in_=wv)
            nc.sync.dma_start(out=wqr_sb[:], in_=wqr)
            nc.sync.dma_start(out=wo_all[:], in_=wo)

            # ---------------- filler work units (~2048 PE cycle quanta) ----
            def v_chunk(sc):
                def run():
                    ps = pjp.tile([128, 512], f32, tag="pj")
                    for dc in range(DC):
                        nc.tensor.matmul(ps[:, 0:KVC],
                                         xt_all[:, dc, sc * 128:(sc + 1) * 128],
                                         wv_all[:, dc, :],
                                         start=(dc == 0), stop=(dc == DC - 1))
                    yield 2048
                    nc.vector.tensor_copy(vv[:, sc, 0:64], ps[:, 0:64])
                    nc.vector.tensor_copy(vv[:, sc, 65:129], ps[:, 64:128])
                return run

            def q_half(qm, jbb, h):
                def run():
                    ps = pjp.tile([128, 512], f32, tag="pj")
                    sl = slice(jbb * 1024 + h * 512, jbb * 1024 + (h + 1) * 512)
                    for dq in range(0, DC, 4):
                        for dc in range(dq, dq + 4):
                            nc.tensor.matmul(ps[:], wq_sl(qm, dc),
                                             xt_all[:, dc, sl],
                                             start=(dc == 0), stop=(dc == DC - 1))
                        yield 2048
                    nc.vector.tensor_copy(qpair[:, qm, sl], ps[:])
                return run

            def o_piece(sm, pc):
                def run():
                    ps = pjp.tile([128, 512], f32, tag="pj")
                    for cc in range(4):
                        nc.tensor.matmul(ps[:], at[:, cc, sm * 128:(sm + 1) * 128],
                                         wo_all[:, cc, pc * 512:(pc + 1) * 512],
                                         start=(cc == 0), stop=(cc == 3))
                    yield 2048
                    o_sb = ev.tile([128, 512], f32, tag="o_sb")
                    nc.vector.tensor_copy(o_sb[:], ps[:])
                    rs = slice(sm * 128, (sm + 1) * 128)
                    nc.sync.dma_start(out=out[rs, pc * 512:(pc + 1) * 512], in_=o_sb[:])
                return run

            # deadline-aware FIFO of filler generators
            queue = []            # [name, ...]
            gens = {}             # name -> generator factory (unstarted)
            started = {}          # name -> running generator
            budget = [0]

            def push(name, factory):
                queue.append(name)
                gens[name] = factory

            def _resume(name):
                g = started.get(name)
                if g is None:
                    g = started[name] = gens.pop(name)()
                try:
                    return next(g)
                except StopIteration:
                    del started[name]
                    queue.remove(name)
                    return None

            def fill(cycles):
                budget[0] += cycles
                while budget[0] > 0 and queue:
                    cost = _resume(queue[0])
                    if cost is not None:
                        budget[0] -= cost

            def require(name):
                # force a unit to finish emission now (deadline)
                while name in queue:
                    _resume(name)

            def drain_all():
                while queue:
                    _resume(queue[0])

            for sc in range(3):
                for _ in v_chunk(sc)():
                    pass
            for sc in range(3, SC):
                push(f"v{sc}", v_chunk(sc))
            push("q1h0", q_half(1, 0, 0))
            push("q1h1", q_half(1, 0, 1))
            releases = {
                0: [("q2h0", q_half(2, 0, 0)), ("q2h1", q_half(2, 0, 1))],
                1: [("q3h0", q_half(3, 0, 0)), ("q3h1", q_half(3, 0, 1))],
                2: [("q4h0", q_half(0, 1, 0)), ("q4h1", q_half(0, 1, 1))],
                3: [("q5h0", q_half(1, 1, 0)), ("q5h1", q_half(1, 1, 1))],
                4: [("q6h0", q_half(2, 1, 0)), ("q6h1", q_half(2, 1, 1)),
                    ("q7h0", q_half(3, 1, 0)), ("q7h1", q_half(3, 1, 1))],
            }

            # packed AV accumulator slots: 18 x [128, 65] f32 in 3 PSUM banks
            def o_slot(t, s, lo, hi):
                b, i = s // 6, s % 6
                return t[:, b, 85 * i + lo:85 * i + hi]

            def av_half(o_all, kc, p, kv, h2):
                for qt in range(8):
                    dst = o_slot(o_all, h2 * 8 + qt, 0, 65)
                    nc.tensor.matmul(dst, p[:, qt * 128:(qt + 1) * 128],
                                     vv[:, kc, kv * 65:kv * 65 + 65],
                                     start=(kc == 0), stop=(kc == SC - 1))

            def nt_unit(o_st, qm, jbb, qt):
                # normalize one q-tile of both heads (DVE) + transpose (PE)
                def run():
                    at_n2 = ev.tile([128, 128], bf16, tag="at_n2")
                    for h2 in range(2):
                        num = o_slot(o_st, h2 * 8 + qt, 0, 64)
                        den = o_slot(o_st, h2 * 8 + qt, 64, 65)
                        rden = dpool.tile([128, 1], f32, tag="rden")
                        nc.vector.reciprocal(rden[:], den)
                        nc.vector.tensor_scalar_mul(
                            at_n2[:, h2 * 64:(h2 + 1) * 64], num, rden[:])
                    tp = pjp.tile([128, 128], bf16, tag="pj")
                    nc.tensor.matmul(tp[:], at_n2[:], ident[:], is_transpose=True)
                    nc.vector.tensor_copy(
                        at[:, qm, jbb * 1024 + qt * 128:jbb * 1024 + (qt + 1) * 128],
                        tp[:])
                    yield 512
                return run

            # ------------- fused attention, per (jb, qm) head pair ----------
            for jb in range(2):
                qsl = slice(jb * 1024, (jb + 1) * 1024)
                for qm in range(4):
                    pair = jb * 4 + qm
                    for name, factory in releases.get(pair, []):
                        push(name, factory)
                    require(f"q{pair}h0")
                    require(f"q{pair}h1")
                    kv = qm // 2
                    o_all = oap.tile([128, 3, 512], f32, tag="o_all")
                    prev = None
                    for kc in range(SC):
                        # per-head interleave: scores for head X at kc only
                        # sit behind AV matmuls that consume the SAME head's
                        # exp from kc-1 (long done), so ACT never drains
                        if prev is not None:
                            require(f"v{prev[0]}")
                            av_half(o_all, prev[0], prev[1], kv, 0)
                        ksl = slice(kc * 128, (kc + 1) * 128)
                        ps_A = sps.tile([128, 1024], f32, tag="s_ps")
                        mmacc(ps_A, ktd[0:64, kv, ksl],
                              qpair[0:64, qm, qsl], 1024, True, True)
                        p_A = ppool.tile([128, 1024], bf16, tag="p_A")
                        nc.scalar.activation(p_A[:], ps_A[:], Exp, scale=SCALE)
                        if prev is not None:
                            av_half(o_all, prev[0], prev[2], kv, 1)
                        ps_B = sps.tile([128, 1024], f32, tag="s_ps")
                        mmacc(ps_B, ktd[64:128, kv, ksl],
                              qpair[64:128, qm, qsl], 1024, True, True)
                        p_B = ppool.tile([128, 1024], bf16, tag="p_B")
                        nc.scalar.activation(p_B[:], ps_B[:], Exp, scale=SCALE)
                        fill(3072)
                        prev = (kc, p_A, p_B)
                    require(f"v{prev[0]}")
                    av_half(o_all, prev[0], prev[1], kv, 0)
                    av_half(o_all, prev[0], prev[2], kv, 1)

                    # free the AV psum banks right away; normalize+transpose
                    # run from the SBUF staging copy, spread over the next
                    # pair's filler slots
                    o_st = ev.tile([128, 3, 512], f32, tag="o_st")
                    nc.vector.tensor_copy(o_st[:], o_all[:])
                    for qt in range(8):
                        push(f"nt{pair}_{qt}", nt_unit(o_st, qm, jb, qt))

                # after jb=0's pairs, at[:, :, 0:1024] completes during pair
                # 4's nt units: release its o_proj pieces from pair 4 on
                # (o_piece(sm) only reads at[:, :, sm*128:...], whose nt unit
                # qt=sm lands before the piece pops from the queue)
                if jb == 0:
                    rel = releases.setdefault(4, [])
                    for sm in range(8):
                        for pc in range(4):
                            rel.append((f"o{sm}_{pc}", o_piece(sm, pc)))

            # ------------- epilogue -----------------------------------------
            # drain remaining queue (includes the last pairs' normalize+
            # transpose units), then jb=1's o_proj on the freed scores pool
            drain_all()
            for sm in range(8, 16):
                for nb in range(2):
                    ps = sps.tile([128, 1024], f32, tag="s_ps")
                    for cc in range(4):
                        mmacc(ps, at[:, cc, sm * 128:(sm + 1) * 128],
                              wo_all[:, cc, nb * 1024:(nb + 1) * 1024], 1024,
                              (cc == 0), (cc == 3))
                    o_sb = ev.tile([128, 1024], f32, tag="o_sb2", bufs=4)
                    nc.vector.tensor_copy(o_sb[:], ps[:])
                    rs = slice(sm * 128, (sm + 1) * 128)
                    nc.sync.dma_start(out=out[rs, nb * 1024:(nb + 1) * 1024],
                                      in_=o_sb[:])

    nc.compile()
    return nc


def _get_nc():
    if "nc" not in _CACHE:
        _CACHE["nc"] = _build()
    return _CACHE["nc"]


def kernel(x, wq, wk, wv, wo):
    from concourse.bass_utils import run_bass_kernel_spmd

    bf16 = ml_dtypes.bfloat16
    nc = _get_nc()

    def chunk_d(a):
        # [D, n] -> [128, DC, n]: partition-major contraction chunks
        n = a.shape[1]
        return np.ascontiguousarray(
            a.reshape(DC, 128, n).transpose(1, 0, 2)).astype(bf16)

    in_maps = []
    for core in range(8):
        b, g = core // 4, core % 4
        wq_g = np.asarray(wq)[:, g * QC:(g + 1) * QC]
        wo_g = np.asarray(wo)[g * QC:(g + 1) * QC, :]
        in_maps.append({
            "xt": np.ascontiguousarray(np.asarray(x)[b].T).astype(bf16),
            "wq0": chunk_d(wq_g[:, 0:128]),
            "wqr": chunk_d(wq_g[:, 128:QC]),
            "wk": chunk_d(np.asarray(wk)[:, g * KVC:(g + 1) * KVC]),
            "wv": chunk_d(np.asarray(wv)[:, g * KVC:(g + 1) * KVC]),
            "wo": np.ascontiguousarray(
                wo_g.reshape(QC // 128, 128, D).transpose(1, 0, 2)).astype(bf16),
        })

    res = run_bass_kernel_spmd(nc, in_maps, core_ids=list(range(8)))
    outs = [res.results[c]["out"] for c in range(8)]
    full = np.empty((2, S, D), np.float32)
    full[0] = outs[0] + outs[1] + outs[2] + outs[3]
    full[1] = outs[4] + outs[5] + outs[6] + outs[7]
    return full


# revision 42
# speedup vs baseline: 1.0266x; 1.0228x over previous
"""LlamaAttention (GQA, no mask) on 8 Trainium2 NeuronCores.

Sharding: 8 cores = 2 (batch) x 4 (head groups of 8 heads / 2 KV heads).
Per core (bf16 compute, fp32 accumulation):
  qT  = (x_b @ wq_g)^T            [512, 2048]   (head dims on partitions)
  kTd = (x_b @ wk_g)^T duplicated [128, 2, 2048]
  v   = x_b @ wv_g (+ ones col)   [2048, 2, 65]
  attention per head pair: sT[k,q] matmuls -> exp on ACT -> flipped AV
    matmuls out[q-tile 128, 65] (full-M: half the PE streaming of the
    [65, q] orientation) accumulated in a packed 3-bank PSUM tile ->
    bulk copy to SBUF -> per-partition reciprocal + mul normalize -> PE
    transpose (identity matmul) back to at^T[d, q] for o_proj.
  out_partial = at @ wo_g         [2048, 2048] fp32
Host sums the 4 head-group partials per batch.

Scheduling: the exp stream on ACT (256 x [128,1024], ~266us) is the body
pacer; every other PE matmul (v/q/o projections, k second half) is
emitted as ~2048-cycle quanta inside the attention kc loop so the PE
array fills ACT-wait slack. Normalize+transpose of pair p is emitted
inside pair p+1's kc loop. xt chunk DMAs are split 4-way so early
contraction chunks land early and the prologue projections chase DMA
arrivals.
"""

import numpy as np
import ml_dtypes

S = 2048          # sequence length
D = 2048          # model dim
HD = 64           # head dim
GH = 8            # heads per core
QC = GH * HD      # 512 q cols per core
KVC = 128         # kv cols per core (2 kv heads)
DC = D // 128     # 16 contraction chunks
SC = S // 128     # 16 seq chunks
SCALE = HD ** -0.5

_CACHE = {}


def _build():
    import concourse.bass as bass
    import concourse.mybir as mybir
    import concourse.tile as tile
    from concourse import bacc, masks

    f32 = mybir.dt.float32
    bf16 = mybir.dt.bfloat16
    Exp = mybir.ActivationFunctionType.Exp

    nc = bacc.Bacc("TRN2", target_bir_lowering=False, debug=False, num_devices=8)

    # weights come host-pre-chunked to partition-major [128, dc, n] layouts
    # so every DMA moves >=4KB contiguous runs (half-rate below 512B); wq is
    # split so pair 0's columns can be prioritized on the serial DMA bus
    xt = nc.dram_tensor("xt", [D, S], bf16, kind="ExternalInput").ap()
    wq0 = nc.dram_tensor("wq0", [128, DC, 128], bf16, kind="ExternalInput").ap()
    wqr = nc.dram_tensor("wqr", [128, DC, 384], bf16, kind="ExternalInput").ap()
    wk = nc.dram_tensor("wk", [128, DC, KVC], bf16, kind="ExternalInput").ap()
    wv = nc.dram_tensor("wv", [128, DC, KVC], bf16, kind="ExternalInput").ap()
    wo = nc.dram_tensor("wo", [128, QC // 128, D], bf16, kind="ExternalInput").ap()
    out = nc.dram_tensor("out", [S, D], f32, kind="ExternalOutput").ap()

    with tile.TileContext(nc) as tc:
        with tc.tile_pool(name="const", bufs=1) as const, \
             tc.tile_pool(name="sps", bufs=2, space="PSUM") as sps, \
             tc.tile_pool(name="pjp", bufs=1, space="PSUM") as pjp, \
             tc.tile_pool(name="oap", bufs=1, space="PSUM") as oap, \
             tc.tile_pool(name="ev", bufs=2) as ev, \
             tc.tile_pool(name="ppool", bufs=3) as ppool, \
             tc.tile_pool(name="dpool", bufs=2) as dpool:

            # resident inputs, issued on ONE queue (SP) in priority order so
            # the serial DMA bus delivers exactly what the prologue needs
            # first: wk + wq(pair0 cols) -> xt chunks (the k/q projections
            # chase the per-chunk arrivals) -> wv -> wq rest -> wo
            wk_all = const.tile([128, DC, KVC], bf16, tag="wk_all")
            nc.sync.dma_start(out=wk_all[:], in_=wk)
            wq0_sb = const.tile([128, DC, 128], bf16, tag="wq0_sb")
            nc.sync.dma_start(out=wq0_sb[:], in_=wq0)
            xt_all = const.tile([128, DC, S], bf16, tag="xt_all")
            xt_re = xt.rearrange("(c p) s -> p c s", p=128)
            for dc in range(DC):
                nc.sync.dma_start(out=xt_all[:, dc, :], in_=xt_re[:, dc, :])
            # wv/wqr/wo DMAs are issued AFTER the ktd dup DMAs below so the
            # dups (needed by the first scores matmul) aren't stuck behind
            # them on the serial DMA bus
            wv_all = const.tile([128, DC, KVC], bf16, tag="wv_all")
            wqr_sb = const.tile([128, DC, 384], bf16, tag="wqr_sb")
            wo_all = const.tile([128, QC // 128, D], bf16, tag="wo_all")

            def wq_sl(qm, dc):
                if qm == 0:
                    return wq0_sb[:, dc, :]
                return wqr_sb[:, dc, (qm - 1) * 128:qm * 128]

            ident = const.tile([128, 128], bf16, tag="ident")
            masks.make_identity(nc, ident[:])

            # persistent intermediates
            qpair = const.tile([128, 4, S], bf16, tag="qpair")     # q^T
            ktd = const.tile([128, 2, S], bf16, tag="ktd")         # k^T dup per kv head
            vv = const.tile([128, SC, 130], bf16, tag="vv")        # v (+ones cols)
            at = const.tile([128, 4, S], bf16, tag="at")           # attn out^T

            nc.vector.memset(vv[:, :, 64:65], 1.0)
            nc.vector.memset(vv[:, :, 129:130], 1.0)

            def mmacc(out_t, lhsT, rhs, width, start, stop):
                # moving-operand ISA limit is 512: split wide matmuls
                for o in range(0, width, 512):
                    nc.tensor.matmul(out_t[:, o:o + 512], lhsT,
                                     rhs[:, o:o + 512], start=start, stop=stop)

            # ---------------- prologue: k + q(pair0) chase the xt DMAs -----
            # k nb0/nb1 in the two sps slots, q pair0 half0 in pj, half1 in
            # the (otherwise idle) o_all psum: all four accumulate per-dc as
            # the xt chunks land.
            k_ps = [sps.tile([128, 1024], f32, tag="s_ps", name=f"k_ps{nb}")
                    for nb in range(2)]
            q_pj = pjp.tile([128, 512], f32, tag="pj")
            o_pro = oap.tile([128, 3, 512], f32, tag="o_all")
            for dc in range(DC):
                for nb in range(2):
                    mmacc(k_ps[nb], wk_all[:, dc, :],
                          xt_all[:, dc, nb * 1024:(nb + 1) * 1024], 1024,
                          (dc == 0), (dc == DC - 1))
                nc.tensor.matmul(q_pj[:], wq_sl(0, dc),
                                 xt_all[:, dc, 0:512],
                                 start=(dc == 0), stop=(dc == DC - 1))
                nc.tensor.matmul(o_pro[:, 0, :], wq_sl(0, dc),
                                 xt_all[:, dc, 512:1024],
                                 start=(dc == 0), stop=(dc == DC - 1))
            for nb in range(2):
                kt_sb = ev.tile([128, 1024], bf16, tag="kt_sb")
                nc.vector.tensor_copy(kt_sb[:], k_ps[nb][:])
                sl = slice(nb * 1024, (nb + 1) * 1024)
                nc.sync.dma_start(out=ktd[0:64, 0, sl], in_=kt_sb[0:64, :])
                nc.sync.dma_start(out=ktd[64:128, 0, sl], in_=kt_sb[0:64, :])
                nc.sync.dma_start(out=ktd[0:64, 1, sl], in_=kt_sb[64:128, :])
                nc.sync.dma_start(out=ktd[64:128, 1, sl], in_=kt_sb[64:128, :])
            nc.vector.tensor_copy(qpair[:, 0, 0:512], q_pj[:])
            nc.vector.tensor_copy(qpair[:, 0, 512:1024], o_pro[:, 0, :])
            # remaining weights, behind the ktd dups on the bus
            nc.sync.dma_start(out=wv_all[:], in_=wv)
            nc.sync.dma_start(out=wqr_sb[:], in_=wqr)
            nc.sync.dma_start(out=wo_all[:], in_=wo)

            # ---------------- filler work units (~2048 PE cycle quanta) ----
            def v_chunk(sc):
                def run():
                    ps = pjp.tile([128, 512], f32, tag="pj")
                    for dc in range(DC):
                        nc.tensor.matmul(ps[:, 0:KVC],
                                         xt_all[:, dc, sc * 128:(sc + 1) * 128],
                                         wv_all[:, dc, :],
                                         start=(dc == 0), stop=(dc == DC - 1))
                    yield 2048
                    nc.vector.tensor_copy(vv[:, sc, 0:64], ps[:, 0:64])
                    nc.vector.tensor_copy(vv[:, sc, 65:129], ps[:, 64:128])
                return run

            def q_half(qm, jbb, h):
                def run():
                    ps = pjp.tile([128, 512], f32, tag="pj")
                    sl = slice(jbb * 1024 + h * 512, jbb * 1024 + (h + 1) * 512)
                    for dq in range(0, DC, 4):
                        for dc in range(dq, dq + 4):
                            nc.tensor.matmul(ps[:], wq_sl(qm, dc),
                                             xt_all[:, dc, sl],
                                             start=(dc == 0), stop=(dc == DC - 1))
                        yield 2048
                    nc.vector.tensor_copy(qpair[:, qm, sl], ps[:])
                return run

            def o_piece(sm, pc):
                def run():
                    ps = pjp.tile([128, 512], f32, tag="pj")
                    for cc in range(4):
                        nc.tensor.matmul(ps[:], at[:, cc, sm * 128:(sm + 1) * 128],
                                         wo_all[:, cc, pc * 512:(pc + 1) * 512],
                                         start=(cc == 0), stop=(cc == 3))
                    yield 2048
                    o_sb = ev.tile([128, 512], f32, tag="o_sb")
                    nc.vector.tensor_copy(o_sb[:], ps[:])
                    rs = slice(sm * 128, (sm + 1) * 128)
                    nc.sync.dma_start(out=out[rs, pc * 512:(pc + 1) * 512], in_=o_sb[:])
                return run

            # deadline-aware FIFO of filler generators
            queue = []            # [name, ...]
            gens = {}             # name -> generator factory (unstarted)
            started = {}          # name -> running generator
            budget = [0]

            def push(name, factory):
                queue.append(name)
                gens[name] = factory

            def _resume(name):
                g = started.get(name)
                if g is None:
                    g = started[name] = gens.pop(name)()
                try:
                    return next(g)
                except StopIteration:
                    del started[name]
                    queue.remove(name)
                    return None

            def fill(cycles):
                budget[0] += cycles
                while budget[0] > 0 and queue:
                    cost = _resume(queue[0])
                    if cost is not None:
                        budget[0] -= cost

            def require(name):
                # force a unit to finish emission now (deadline)
                while name in queue:
                    _resume(name)

            def drain_all():
                while queue:
                    _resume(queue[0])

            for sc in range(3):
                for _ in v_chunk(sc)():
                    pass
            for sc in range(3, SC):
                push(f"v{sc}", v_chunk(sc))
            push("q1h0", q_half(1, 0, 0))
            push("q1h1", q_half(1, 0, 1))
            releases = {
                0: [("q2h0", q_half(2, 0, 0)), ("q2h1", q_half(2, 0, 1))],
                1: [("q3h0", q_half(3, 0, 0)), ("q3h1", q_half(3, 0, 1))],
                2: [("q4h0", q_half(0, 1, 0)), ("q4h1", q_half(0, 1, 1))],
                3: [("q5h0", q_half(1, 1, 0)), ("q5h1", q_half(1, 1, 1))],
                4: [("q6h0", q_half(2, 1, 0)), ("q6h1", q_half(2, 1, 1)),
                    ("q7h0", q_half(3, 1, 0)), ("q7h1", q_half(3, 1, 1))],
            }

            # packed AV accumulator slots: 18 x [128, 65] f32 in 3 PSUM banks
            def o_slot(t, s, lo, hi):
                b, i = s // 6, s % 6
                return t[:, b, 85 * i + lo:85 * i + hi]

            def av_half(o_all, kc, p, kv, h2):
                for qt in range(8):
                    dst = o_slot(o_all, h2 * 8 + qt, 0, 65)
                    nc.tensor.matmul(dst, p[:, qt * 128:(qt + 1) * 128],
                                     vv[:, kc, kv * 65:kv * 65 + 65],
                                     start=(kc == 0), stop=(kc == SC - 1))

            def nt_unit(o_st, qm, jbb, qt):
                # normalize one q-tile of both heads (DVE) + transpose (PE)
                def run():
                    at_n2 = ev.tile([128, 128], bf16, tag="at_n2")
                    for h2 in range(2):
                        num = o_slot(o_st, h2 * 8 + qt, 0, 64)
                        den = o_slot(o_st, h2 * 8 + qt, 64, 65)
                        rden = dpool.tile([128, 1], f32, tag="rden")
                        nc.vector.reciprocal(rden[:], den)
                        nc.vector.tensor_scalar_mul(
                            at_n2[:, h2 * 64:(h2 + 1) * 64], num, rden[:])
                    tp = pjp.tile([128, 128], bf16, tag="pj")
                    nc.tensor.matmul(tp[:], at_n2[:], ident[:], is_transpose=True)
                    nc.vector.tensor_copy(
                        at[:, qm, jbb * 1024 + qt * 128:jbb * 1024 + (qt + 1) * 128],
                        tp[:])
                    yield 512
                return run

            # ------------- fused attention, per (jb, qm) head pair ----------
            for jb in range(2):
                qsl = slice(jb * 1024, (jb + 1) * 1024)
                for qm in range(4):
                    pair = jb * 4 + qm
                    for name, factory in releases.get(pair, []):
                        push(name, factory)
                    require(f"q{pair}h0")
                    require(f"q{pair}h1")
                    kv = qm // 2
                    o_all = oap.tile([128, 3, 512], f32, tag="o_all")
                    prev = None
                    for kc in range(SC):
                        # scores + exp go FIRST each iteration so ACT's next
                        # instruction is dispatchable the moment the previous
                        # exp retires; the sem-latency-laden AV consumers of
                        # iteration kc-1 run behind them
                        ksl = slice(kc * 128, (kc + 1) * 128)
                        ps_A = sps.tile([128, 1024], f32, tag="s_ps")
                        mmacc(ps_A, ktd[0:64, kv, ksl],
                              qpair[0:64, qm, qsl], 1024, True, True)
                        p_A = ppool.tile([128, 1024], bf16, tag="p_A")
                        nc.scalar.activation(p_A[:], ps_A[:], Exp, scale=SCALE)
                        if prev is not None:
                            require(f"v{prev[0]}")
                            av_half(o_all, prev[0], prev[1], kv, 0)
                        ps_B = sps.tile([128, 1024], f32, tag="s_ps")
                        mmacc(ps_B, ktd[64:128, kv, ksl],
                              qpair[64:128, qm, qsl], 1024, True, True)
                        p_B = ppool.tile([128, 1024], bf16, tag="p_B")
                        nc.scalar.activation(p_B[:], ps_B[:], Exp, scale=SCALE)
                        if prev is not None:
                            av_half(o_all, prev[0], prev[2], kv, 1)
                        fill(3072)
                        prev = (kc, p_A, p_B)
                    require(f"v{prev[0]}")
                    av_half(o_all, prev[0], prev[1], kv, 0)
                    av_half(o_all, prev[0], prev[2], kv, 1)

                    # free the AV psum banks right away; normalize+transpose
                    # run from the SBUF staging copy, spread over the next
                    # pair's filler slots
                    o_st = ev.tile([128, 3, 512], f32, tag="o_st")
                    nc.vector.tensor_copy(o_st[:], o_all[:])
                    for qt in range(8):
                        push(f"nt{pair}_{qt}", nt_unit(o_st, qm, jb, qt))

                # after jb=0's pairs, at[:, :, 0:1024] completes during pair
                # 4's nt units: release its o_proj pieces from pair 4 on
                # (o_piece(sm) only reads at[:, :, sm*128:...], whose nt unit
                # qt=sm lands before the piece pops from the queue)
                if jb == 0:
                    rel = releases.setdefault(4, [])
                    for sm in range(8):
                        for pc in range(4):
                            rel.append((f"o{sm}_{pc}", o_piece(sm, pc)))

            # ------------- epilogue -----------------------------------------
            # drain remaining queue (includes the last pairs' normalize+
            # transpose units), then jb=1's o_proj on the freed scores pool
            drain_all()
            for sm in range(8, 16):
                for nb in range(2):
                    ps = sps.tile([128, 1024], f32, tag="s_ps")
                    for cc in range(4):
                        mmacc(ps, at[:, cc, sm * 128:(sm + 1) * 128],
                              wo_all[:, cc, nb * 1024:(nb + 1) * 1024], 1024,
                              (cc == 0), (cc == 3))
                    o_sb = ev.tile([128, 1024], f32, tag="o_sb2", bufs=4)
                    nc.vector.tensor_copy(o_sb[:], ps[:])
                    rs = slice(sm * 128, (sm + 1) * 128)
                    nc.sync.dma_start(out=out[rs, nb * 1024:(nb + 1) * 1024],
                                      in_=o_sb[:])

    nc.compile()
    return nc


def _get_nc():
    if "nc" not in _CACHE:
        _CACHE["nc"] = _build()
    return _CACHE["nc"]


def kernel(x, wq, wk, wv, wo):
    from concourse.bass_utils import run_bass_kernel_spmd

    bf16 = ml_dtypes.bfloat16
    nc = _get_nc()

    def chunk_d(a):
        # [D, n] -> [128, DC, n]: partition-major contraction chunks
        n = a.shape[1]
        return np.ascontiguousarray(
            a.reshape(DC, 128, n).transpose(1, 0, 2)).astype(bf16)

    in_maps = []
    for core in range(8):
        b, g = core // 4, core % 4
        wq_g = np.asarray(wq)[:, g * QC:(g + 1) * QC]
        wo_g = np.asarray(wo)[g * QC:(g + 1) * QC, :]
        in_maps.append({
            "xt": np.ascontiguousarray(np.asarray(x)[b].T).astype(bf16),
            "wq0": chunk_d(wq_g[:, 0:128]),
            "wqr": chunk_d(wq_g[:, 128:QC]),
            "wk": chunk_d(np.asarray(wk)[:, g * KVC:(g + 1) * KVC]),
            "wv": chunk_d(np.asarray(wv)[:, g * KVC:(g + 1) * KVC]),
            "wo": np.ascontiguousarray(
                wo_g.reshape(QC // 128, 128, D).transpose(1, 0, 2)).astype(bf16),
        })

    res = run_bass_kernel_spmd(nc, in_maps, core_ids=list(range(8)))
    outs = [res.results[c]["out"] for c in range(8)]
    full = np.empty((2, S, D), np.float32)
    full[0] = outs[0] + outs[1] + outs[2] + outs[3]
    full[1] = outs[4] + outs[5] + outs[6] + outs[7]
    return full
